# revision 23
# baseline (speedup 1.0000x reference)
"""Class-balanced cross-entropy loss kernel for Trainium2 (8 NeuronCores).

Problem: output [4,8,64,128,128] f32 logits, labels [4,1,64,128,128] int
(values 0..7).  loss = mean over present classes of (per-class mean CE).

Strategy (data-parallel over the flattened voxel axis, 524288 voxels/core):
  per-voxel CE loss  l_i = logsumexp_c(x_ic) - x_i[lab_i]
  per-class sums     sums[c]  = S_lse[c] - S_g[c]
     S_lse[c] = sum_{i: lab=c} lse_i      (masked accumulate, DVE)
     S_g[c]   = sum_{i: lab=c} x_i[c]     (masked accumulate, DVE)
     counts[c]                            (masked accumulate, DVE)
  final scalar combined on host from tiny per-core partials.

Device layout per core: 4 superblocks of 8 slabs (slab = H*W = 16384 vox).
  x tiles  [128, 4096] bf16, two per superblock (class halves):
     xlo[chat*32+v1, shat*512+v2] = x[b, chat,   d, v1, v2]   chat in 0..3
     xhi[...]                     = x[b, chat+4, d, v1, v2]
  exp on ACT; s = sum over 8 classes via two PE matmuls (G32 stationary
  group-sum matrix, second matmul accumulates with start=False) -> PSUM.
  lse = log(s) on ACT -> per-core [128, 4096] bf16 buffer.
  Masked per-class accumulations via scalar_tensor_tensor / tensor_scalar
  with fused per-partition accum_out (bf16 operands -> 2x/4x DVE modes).
"""

import numpy as np
import ml_dtypes

import concourse.bass as bass
import concourse.bacc as bacc
import concourse.mybir as mybir
from concourse import bass_utils, tile

BF16 = mybir.dt.bfloat16
F32 = mybir.dt.float32
NPBF16 = ml_dtypes.bfloat16

N_CORES = 8
B, C, D, H, W = 4, 8, 64, 128, 128
N_SB = 4                                # superblocks per core
SB_COLS = 4096
VOX_PER_CORE = 32 * H * W               # 524288

_PROG_CACHE = {}


def _build_program():
    nc = bacc.Bacc("TRN2", target_bir_lowering=False, debug=False)

    x_in = nc.dram_tensor("x", [N_SB, 2, 128, SB_COLS], BF16, kind="ExternalInput")
    lr_in = nc.dram_tensor("labrep", [N_SB, 128, SB_COLS], BF16, kind="ExternalInput")
    ll_in = nc.dram_tensor("lablse", [128, SB_COLS], BF16, kind="ExternalInput")
    g32_in = nc.dram_tensor("g32", [128, 32], BF16, kind="ExternalInput")
    pm4_in = nc.dram_tensor("pm4", [128, 2], F32, kind="ExternalInput")
    hbias_in = nc.dram_tensor("hbias", [128, 7], F32, kind="ExternalInput")
    out_d = nc.dram_tensor("partials", [128, 53], F32, kind="ExternalOutput")

    with tile.TileContext(nc) as tc:
        with (
            tc.tile_pool(name="const", bufs=1) as cpool,
            tc.tile_pool(name="io", bufs=3) as iopool,
            tc.tile_pool(name="work", bufs=2) as wpool,
            tc.tile_pool(name="psum", bufs=8, space="PSUM") as ppool,
        ):
            eq = mybir.AluOpType.is_equal
            mul = mybir.AluOpType.mult

            # sb0's first half goes out before everything else so compute
            # can start as early as possible (single HW queue, program order)
            xlo0 = iopool.tile([128, SB_COLS], BF16, tag="xlo")
            lr0 = iopool.tile([128, SB_COLS], BF16, tag="lr")
            xhi0 = iopool.tile([128, SB_COLS], BF16, tag="xhi")
            h0 = slice(0, SB_COLS // 2)
            h1 = slice(SB_COLS // 2, SB_COLS)
            nc.sync.dma_start(xlo0[:, h0], x_in[0, 0][:, h0])
            nc.sync.dma_start(lr0[:, h0], lr_in[0][:, h0])

            g32 = cpool.tile([128, 32], BF16)
            nc.sync.dma_start(g32[:], g32_in[:])
            pm4 = cpool.tile([128, 2], F32)
            nc.sync.dma_start(pm4[:], pm4_in[:])
            hbias = cpool.tile([128, 7], F32)
            nc.sync.dma_start(hbias[:], hbias_in[:])

            nc.sync.dma_start(xhi0[:, h0], x_in[0, 1][:, h0])
            nc.sync.dma_start(xlo0[:, h1], x_in[0, 0][:, h1])
            nc.sync.dma_start(lr0[:, h1], lr_in[0][:, h1])
            nc.sync.dma_start(xhi0[:, h1], x_in[0, 1][:, h1])

            lab_lse = cpool.tile([128, SB_COLS], BF16)
            nc.sync.dma_start(lab_lse[:], ll_in[:])
            lse = cpool.tile([128, SB_COLS], BF16)
            sg_acc = cpool.tile([128, 10], F32)
            slse_acc = cpool.tile([128, 28], F32)
            cnt_acc = cpool.tile([128, 7], F32)
            glse_acc = cpool.tile([128, 8], F32)

            # tiny reads that absorb DMA semaphore waits so wait-slot-limited
            # ops need at most one wait
            dummy = cpool.tile([128, 4], F32)
            nc.vector.tensor_copy(dummy[:, 0:2], pm4[:])
            nc.vector.tensor_copy(dummy[:, 2:3], lab_lse[:, 0:1])
            nc.scalar.activation(
                dummy[:, 3:4], hbias[:, 0:1], mybir.ActivationFunctionType.Copy
            )

            pstiles = []
            for sb in range(N_SB):
                if sb == 0:
                    xlo, xhi, lr_sb = xlo0, xhi0, lr0
                    chunks = (h0, h1)
                else:
                    xlo = iopool.tile([128, SB_COLS], BF16, tag="xlo")
                    xhi = iopool.tile([128, SB_COLS], BF16, tag="xhi")
                    lr_sb = iopool.tile([128, SB_COLS], BF16, tag="lr")
                    nc.sync.dma_start(xlo[:], x_in[sb, 0])
                    nc.sync.dma_start(lr_sb[:], lr_in[sb])
                    nc.sync.dma_start(xhi[:], x_in[sb, 1])
                    chunks = (slice(0, SB_COLS),)
                nc.vector.tensor_copy(dummy[:, 3:4], lr_sb[:, 0:1])

                # S_g partials + exp, per chunk
                elo = wpool.tile([128, SB_COLS], BF16, tag="elo")
                ehi = wpool.tile([128, SB_COLS], BF16, tag="ehi")
                for ci, cs in enumerate(chunks):
                    for h, x_sb in ((0, xlo), (1, xhi)):
                        col = (2 * sb + h) if ci == 0 else (8 + h)
                        sc = wpool.tile([128, SB_COLS], BF16, tag="sc")
                        nc.vector.scalar_tensor_tensor(
                            sc[:, cs],
                            lr_sb[:, cs],
                            pm4[:, h : h + 1],
                            x_sb[:, cs],
                            eq,
                            mul,
                            accum_out=sg_acc[:, col : col + 1],
                        )
                    nc.scalar.activation(
                        elo[:, cs], xlo[:, cs], mybir.ActivationFunctionType.Exp
                    )
                    nc.scalar.activation(
                        ehi[:, cs], xhi[:, cs], mybir.ActivationFunctionType.Exp
                    )

                # class-group sums on PE into per-superblock psum tiles
                for g in range(2):
                    ps = ppool.tile([128, 512], F32, tag="ps")
                    for q in range(4):
                        sl = 512 * (4 * g + q)
                        nc.tensor.matmul(
                            ps[32 * q : 32 * (q + 1), :],
                            g32[:],
                            elo[:, sl : sl + 512],
                            start=True,
                            stop=False,
                            tile_position=(0, 32 * q),
                        )
                        nc.tensor.matmul(
                            ps[32 * q : 32 * (q + 1), :],
                            g32[:],
                            ehi[:, sl : sl + 512],
                            start=False,
                            stop=True,
                            tile_position=(0, 32 * q),
                        )
                    pstiles.append((2 * sb + g, ps))

                # lns + masked-lse passes with granularity shrinking toward
                # the end of the kernel (tail is DVE-bound: keep it short)
                if sb == 1:
                    spans = [(0, 2048, 0)]        # pair {0,1} in one go
                elif sb == 2:
                    spans = [(2048, 1024, 1)]     # sb2
                elif sb == 3:
                    spans = [(3072, 512, 2), (3584, 512, 3)]  # sb3 halves
                else:
                    spans = []
                if spans:
                    for u, ps in pstiles:
                        nc.scalar.activation(
                            lse[:, 512 * u : 512 * (u + 1)],
                            ps[:],
                            mybir.ActivationFunctionType.Ln,
                            accum_out=glse_acc[:, u : u + 1],
                        )
                    pstiles = []
                for off, width, blk in spans:
                    lsl = lse[:, off : off + width]
                    lll = lab_lse[:, off : off + width]
                    for c in range(7):
                        sc2 = wpool.tile([128, 2048], BF16, tag="sc2")
                        nc.vector.scalar_tensor_tensor(
                            sc2[:, 0:width],
                            lll,
                            float(c),
                            lsl,
                            eq,
                            mul,
                            accum_out=slse_acc[:, 7 * blk + c : 7 * blk + c + 1],
                        )

            # count functionals at the end (ACT's tail):
            # m_j = sum_i sign(lab_i - j + 0.5) = N - 2*cum_count(j);
            # sign is exact (+-1) and lives in every ACT table set.
            for j in range(7):
                sc3 = wpool.tile([128, SB_COLS], BF16, tag="sc3")
                nc.scalar.activation(
                    sc3[:],
                    lab_lse[:],
                    mybir.ActivationFunctionType.Sign,
                    bias=hbias[:, j : j + 1],
                    accum_out=cnt_acc[:, j : j + 1],
                )

            nc.sync.dma_start(out_d[:, 0:10], sg_acc[:])
            nc.sync.dma_start(out_d[:, 10:38], slse_acc[:])
            nc.sync.dma_start(out_d[:, 38:45], cnt_acc[:])
            nc.sync.dma_start(out_d[:, 45:53], glse_acc[:])

    nc.compile()
    return nc


def _host_prep(output, labels):
    """Build per-core input maps (sharding + layout prep, no math)."""
    x = np.asarray(output)
    lab = np.asarray(labels).astype(np.int32)

    g32 = np.zeros((128, 32), dtype=NPBF16)
    for ch in range(4):
        for v1 in range(32):
            g32[ch * 32 + v1, v1] = 1.0
    pcls = np.arange(128, dtype=np.int32) // 32
    pm4 = np.stack([pcls, pcls + 4], axis=1).astype(np.float32)

    in_maps = []
    for k in range(N_CORES):
        b, d0 = k // 2, 32 * (k % 2)
        # [8c, 4sb, 8shat, 32v1, 512v2] -> [sb, chat, v1, shat, v2]
        xc = x[b, :, d0 : d0 + 32].reshape(8, 4, 8, 32, 512)
        xt = xc.transpose(1, 0, 3, 2, 4).astype(NPBF16)  # [sb, c, v1, shat, v2]
        x_prep = np.stack(
            [
                np.ascontiguousarray(xt[:, :4]).reshape(4, 128, 4096),
                np.ascontiguousarray(xt[:, 4:]).reshape(4, 128, 4096),
            ],
            axis=1,
        )

        lc = lab[b, 0, d0 : d0 + 32].reshape(4, 8, 32, 512).astype(NPBF16)
        # labrep[sb, chat*32+v1, shat*512+v2]
        lr = lc.transpose(0, 2, 1, 3).reshape(4, 1, 32, 4096)
        lr = np.ascontiguousarray(
            np.broadcast_to(lr, (4, 4, 32, 4096))
        ).reshape(4, 128, 4096)
        # lablse[(shat%4)*32+v1, (2*sb + shat//4)*512+v2]
        l2 = lc.reshape(4, 2, 4, 32, 512)  # [sb, sh, sl, v1, v2]
        ll = np.ascontiguousarray(l2.transpose(2, 3, 0, 1, 4)).reshape(128, 4096)

        in_maps.append(
            {
                "x": x_prep,
                "labrep": lr,
                "lablse": ll,
                "g32": g32,
                "pm4": pm4,
                "hbias": np.broadcast_to(
                    0.5 - np.arange(1, 8, dtype=np.float32), (128, 7)
                ).copy(),
            }
        )
    return in_maps


def _combine(results):
    """Host gather: reduce per-core [3,128,8] partials to the final scalar."""
    S_g = np.zeros(8, dtype=np.float64)
    S_lse = np.zeros(8, dtype=np.float64)
    cnt = np.zeros(8, dtype=np.float64)
    pclass = np.arange(128) // 32  # 0..3 per partition
    m = np.zeros(7, dtype=np.float64)
    glse = 0.0
    n_total = 0
    for r in results:
        p = np.asarray(r["partials"], dtype=np.float64)
        sg, slse, cn = p[:, 0:10], p[:, 10:38], p[:, 38:45]
        lo_cols, hi_cols = [0, 2, 4, 6, 8], [1, 3, 5, 7, 9]
        for ch in range(4):
            rows = pclass == ch
            S_g[ch] += sg[np.ix_(rows, lo_cols)].sum()
            S_g[ch + 4] += sg[np.ix_(rows, hi_cols)].sum()
        sl = slse.sum(axis=0).reshape(4, 7).sum(axis=0)
        S_lse[:7] += sl
        glse += p[:, 45:53].sum()
        m += cn.sum(axis=0)
        n_total += VOX_PER_CORE
    S_lse[7] = glse - S_lse[:7].sum()
    # histogram from sign-staircase functionals (exact +-1 entries)
    js = np.arange(1, 8, dtype=np.float64)
    A = np.vstack(
        [np.ones(8), np.sign(np.arange(8)[None, :] - js[:, None] + 0.5)]
    )
    cnt[:] = np.round(np.linalg.solve(A, np.concatenate([[n_total], m])))
    sums = S_lse - S_g
    present = cnt > 0
    class_means = sums / np.maximum(cnt, 1.0)
    n_valid = present.sum()
    loss = np.where(present, class_means, 0.0).sum() / n_valid
    return np.float32(loss)


def run(inputs_maps=None, trace=False, **inputs):
    if "nc" not in _PROG_CACHE:
        _PROG_CACHE["nc"] = _build_program()
    nc = _PROG_CACHE["nc"]
    in_maps = inputs_maps if inputs_maps is not None else _host_prep(**inputs)
    res = bass_utils.run_bass_kernel_spmd(
        nc, in_maps, list(range(N_CORES)), trace=trace
    )
    return res


def kernel(output, labels):
    res = run(output=output, labels=labels)
    return _combine(res.results)


# revision 25
# speedup vs baseline: 1.1317x; 1.1317x over previous
"""Class-balanced cross-entropy loss kernel for Trainium2 (8 NeuronCores).

Problem: output [4,8,64,128,128] f32 logits, labels [4,1,64,128,128] int
(values 0..7).  loss = mean over present classes of (per-class mean CE).

Strategy (data-parallel over the flattened voxel axis, 524288 voxels/core):
  per-voxel CE loss  l_i = logsumexp_c(x_ic) - x_i[lab_i]
  per-class sums     sums[c]  = S_lse[c] - S_g[c]
     S_g[c]   = sum_{i: lab=c} x_i[c]
     S_lse[c] = sum_{i: lab=c} lse_i
     counts[c]
  final scalar combined on host from tiny per-core partials.

Inputs are pre-laid-out on host (sharding + one-hot label encoding only):
  x tiles   [4sb][2half][128, 4096] bf16, partition p = chat*32+v1,
            free f = shat*512+v2  (chat = class within half)
  onehot_x  same layout: 1.0 where lab == class(p) else 0
  lab_lse   [128, 4096] bf16 labels in the lse layout
Masked products run on the DVE at the 2x bf16 tensor_tensor rate; all
reductions run on the TensorEngine (PSUM-accumulating matmuls with
stationary selector matrices) or ride free activation accum_outs:
    s      = sum_c exp(x)      via G32 group-sum matmuls -> PSUM
    lse    = ln(s) on ACT (free accum_out -> global lse sum)
    S_g    = SEL^T @ (onehot_x * x)      accumulated in PSUM [8,512]
    S_lse  = E_c^T @ ((lab==c) * lse)    accumulated in PSUM [8,512]
    counts = E_c^T @ (lab==c)            accumulated in PSUM [8,512]
"""

import numpy as np
import ml_dtypes

import concourse.bass as bass
import concourse.bacc as bacc
import concourse.mybir as mybir
from concourse import bass_utils, tile

BF16 = mybir.dt.bfloat16
F32 = mybir.dt.float32
NPBF16 = ml_dtypes.bfloat16

N_CORES = 8
B, C, D, H, W = 4, 8, 64, 128, 128
N_SB = 4
SB_COLS = 4096
VOX_PER_CORE = 32 * H * W  # 524288

_PROG_CACHE = {}


def _build_program():
    nc = bacc.Bacc("TRN2", target_bir_lowering=False, debug=False)

    x_in = nc.dram_tensor("x", [N_SB, 2, 128, SB_COLS], BF16, kind="ExternalInput")
    oh_in = nc.dram_tensor("onehot", [N_SB, 2, 128, SB_COLS], BF16, kind="ExternalInput")
    ll_in = nc.dram_tensor("lablse", [128, SB_COLS], BF16, kind="ExternalInput")
    g32_in = nc.dram_tensor("g32", [128, 32], BF16, kind="ExternalInput")
    sel_in = nc.dram_tensor("sel", [128, 16], BF16, kind="ExternalInput")
    ecol_in = nc.dram_tensor("ecol", [128, 56], BF16, kind="ExternalInput")
    out_d = nc.dram_tensor("partials", [128, 11], F32, kind="ExternalOutput")

    eq = mybir.AluOpType.is_equal
    mul = mybir.AluOpType.mult

    with tile.TileContext(nc) as tc:
        with (
            tc.tile_pool(name="const", bufs=1) as cpool,
            tc.tile_pool(name="io", bufs=3) as iopool,
            tc.tile_pool(name="work", bufs=2) as wpool,
            tc.tile_pool(name="psum", bufs=4, space="PSUM") as ppool,
            tc.tile_pool(name="psacc", bufs=1, space="PSUM") as papool,
        ):
            # sb0's first half goes out before everything else
            xlo0 = iopool.tile([128, SB_COLS], BF16, tag="xlo")
            olo0 = iopool.tile([128, SB_COLS], BF16, tag="olo")
            xhi0 = iopool.tile([128, SB_COLS], BF16, tag="xhi")
            ohi0 = iopool.tile([128, SB_COLS], BF16, tag="ohi")
            h0 = slice(0, SB_COLS // 2)
            h1 = slice(SB_COLS // 2, SB_COLS)
            nc.sync.dma_start(xlo0[:, h0], x_in[0, 0][:, h0])
            nc.sync.dma_start(olo0[:, h0], oh_in[0, 0][:, h0])

            g32 = cpool.tile([128, 32], BF16)
            nc.sync.dma_start(g32[:], g32_in[:])
            sel = cpool.tile([128, 16], BF16)
            nc.sync.dma_start(sel[:], sel_in[:])
            ecol = cpool.tile([128, 56], BF16)
            nc.sync.dma_start(ecol[:], ecol_in[:])

            nc.sync.dma_start(xhi0[:, h0], x_in[0, 1][:, h0])
            nc.sync.dma_start(ohi0[:, h0], oh_in[0, 1][:, h0])
            nc.sync.dma_start(xlo0[:, h1], x_in[0, 0][:, h1])
            nc.sync.dma_start(olo0[:, h1], oh_in[0, 0][:, h1])
            nc.sync.dma_start(xhi0[:, h1], x_in[0, 1][:, h1])
            nc.sync.dma_start(ohi0[:, h1], oh_in[0, 1][:, h1])

            lab_lse = cpool.tile([128, SB_COLS], BF16)
            nc.sync.dma_start(lab_lse[:], ll_in[:])
            lse = cpool.tile([128, SB_COLS], BF16)
            glse_acc = cpool.tile([128, 8], F32)
            final = cpool.tile([8, 3], F32)

            # psum accumulators alive for the whole kernel
            ps_sg = papool.tile([8, 512], F32)
            ps_cnt = papool.tile([8, 512], F32)
            ps_slse = papool.tile([8, 512], F32)

            # tiny reads that absorb DMA semaphore waits
            dummy = cpool.tile([128, 4], F32)
            nc.vector.tensor_copy(dummy[:, 0:1], g32[:, 0:1])
            nc.vector.tensor_copy(dummy[:, 1:2], lab_lse[:, 0:1])
            nc.vector.tensor_copy(dummy[:, 2:3], sel[:, 0:1])
            nc.vector.tensor_copy(dummy[:, 3:4], ecol[:, 0:1])

            first_sg = [True]
            first_cls = [True]

            def sg_matmuls(m, h, cols):
                lhs = sel[:, 8 * h : 8 * h + 8]
                for ci in range(cols.start // 512, cols.stop // 512):
                    nc.tensor.matmul(
                        ps_sg[:, :],
                        lhs,
                        m[:, 512 * ci : 512 * (ci + 1)],
                        start=first_sg[0],
                        stop=False,
                        skip_group_check=True,
                    )
                    first_sg[0] = False

            pstiles = []
            for sb in range(N_SB):
                if sb == 0:
                    xlo, xhi, olo, ohi = xlo0, xhi0, olo0, ohi0
                    chunks = (h0, h1)
                else:
                    xlo = iopool.tile([128, SB_COLS], BF16, tag="xlo")
                    olo = iopool.tile([128, SB_COLS], BF16, tag="olo")
                    xhi = iopool.tile([128, SB_COLS], BF16, tag="xhi")
                    ohi = iopool.tile([128, SB_COLS], BF16, tag="ohi")
                    nc.sync.dma_start(xlo[:], x_in[sb, 0])
                    nc.sync.dma_start(olo[:], oh_in[sb, 0])
                    nc.sync.dma_start(xhi[:], x_in[sb, 1])
                    nc.sync.dma_start(ohi[:], oh_in[sb, 1])
                    chunks = (slice(0, SB_COLS),)

                nc.vector.tensor_copy(dummy[:, 0:1], olo[:, 0:1])
                nc.vector.tensor_copy(dummy[:, 1:2], ohi[:, 0:1])

                elo = wpool.tile([128, SB_COLS], BF16, tag="elo")
                ehi = wpool.tile([128, SB_COLS], BF16, tag="ehi")
                mlo = wpool.tile([128, SB_COLS], BF16, tag="mlo")
                mhi = wpool.tile([128, SB_COLS], BF16, tag="mhi")
                for cs in chunks:
                    nc.vector.tensor_tensor(mlo[:, cs], olo[:, cs], xlo[:, cs], mul)
                    sg_matmuls(mlo, 0, cs)
                    nc.scalar.activation(
                        elo[:, cs], xlo[:, cs], mybir.ActivationFunctionType.Exp
                    )
                    nc.vector.tensor_tensor(mhi[:, cs], ohi[:, cs], xhi[:, cs], mul)
                    sg_matmuls(mhi, 1, cs)
                    nc.scalar.activation(
                        ehi[:, cs], xhi[:, cs], mybir.ActivationFunctionType.Exp
                    )

                # softmax denominator: class-group sums on PE
                for g in range(2):
                    ps = ppool.tile([128, 512], F32, tag="ps")
                    for q in range(4):
                        sl = 512 * (4 * g + q)
                        nc.tensor.matmul(
                            ps[32 * q : 32 * (q + 1), :],
                            g32[:],
                            elo[:, sl : sl + 512],
                            start=True,
                            stop=False,
                            tile_position=(0, 32 * q),
                        )
                        nc.tensor.matmul(
                            ps[32 * q : 32 * (q + 1), :],
                            g32[:],
                            ehi[:, sl : sl + 512],
                            start=False,
                            stop=True,
                            tile_position=(0, 32 * q),
                        )
                    pstiles.append((2 * sb + g, ps))

                if sb % 2 == 0:
                    continue

                # end of pair: batched lns (fused global-lse accum), then
                # per-class masked lse products + counts, reduced on PE
                pair = sb // 2
                for u, ps in pstiles:
                    nc.scalar.activation(
                        lse[:, 512 * u : 512 * (u + 1)],
                        ps[:],
                        mybir.ActivationFunctionType.Ln,
                        accum_out=glse_acc[:, u : u + 1],
                    )
                pstiles = []
                pcs = slice(2048 * pair, 2048 * (pair + 1))
                for c in range(7):
                    ohc = wpool.tile([128, 2048], BF16, tag="ohc")
                    nc.vector.tensor_scalar(
                        ohc[:], lab_lse[:, pcs], float(c), None, eq
                    )
                    mls = wpool.tile([128, 2048], BF16, tag="mls")
                    nc.vector.tensor_tensor(mls[:], ohc[:], lse[:, pcs], mul)
                    lhs = ecol[:, 8 * c : 8 * c + 8]
                    for ci in range(4):
                        nc.tensor.matmul(
                            ps_cnt[:, :],
                            lhs,
                            ohc[:, 512 * ci : 512 * (ci + 1)],
                            start=first_cls[0],
                            stop=False,
                            skip_group_check=True,
                        )
                        nc.tensor.matmul(
                            ps_slse[:, :],
                            lhs,
                            mls[:, 512 * ci : 512 * (ci + 1)],
                            start=first_cls[0],
                            stop=False,
                            skip_group_check=True,
                        )
                        first_cls[0] = False

            # fold the [8, 512] psum accumulators to [8, 1]
            nc.vector.tensor_reduce(
                final[0:8, 0:1], ps_sg[:], mybir.AxisListType.X, mybir.AluOpType.add
            )
            nc.vector.tensor_reduce(
                final[0:8, 1:2], ps_cnt[:], mybir.AxisListType.X, mybir.AluOpType.add
            )
            nc.vector.tensor_reduce(
                final[0:8, 2:3], ps_slse[:], mybir.AxisListType.X, mybir.AluOpType.add
            )

            nc.sync.dma_start(out_d[:, 0:8], glse_acc[:])
            nc.sync.dma_start(out_d[0:8, 8:11], final[0:8, 0:3])

    nc.compile()
    return nc


def _host_prep(output, labels):
    """Build per-core input maps (sharding + layout/encoding prep)."""
    x = np.asarray(output)
    lab = np.asarray(labels).astype(np.int32)

    g32 = np.zeros((128, 32), dtype=NPBF16)
    for ch in range(4):
        for v1 in range(32):
            g32[ch * 32 + v1, v1] = 1.0
    sel = np.zeros((128, 16), dtype=NPBF16)
    for p in range(128):
        sel[p, p // 32] = 1.0            # lo half -> classes 0..3
        sel[p, 8 + 4 + p // 32] = 1.0    # hi half -> classes 4..7
    ecol = np.zeros((128, 56), dtype=NPBF16)
    for c in range(7):
        ecol[:, 8 * c + c] = 1.0

    in_maps = []
    for k in range(N_CORES):
        b, d0 = k // 2, 32 * (k % 2)
        xc = x[b, :, d0 : d0 + 32].reshape(8, 4, 8, 32, 512)
        xt = xc.transpose(1, 0, 3, 2, 4).astype(NPBF16)  # [sb, c, v1, shat, v2]
        x_prep = np.stack(
            [
                np.ascontiguousarray(xt[:, :4]).reshape(4, 128, 4096),
                np.ascontiguousarray(xt[:, 4:]).reshape(4, 128, 4096),
            ],
            axis=1,
        )

        lc = lab[b, 0, d0 : d0 + 32].reshape(4, 8, 32, 512)
        # one-hot label encoding in the x layout: [sb, cls, v1, shat, v2]
        lt = lc.transpose(0, 2, 1, 3)[:, None]           # [sb, 1, v1, shat, v2]
        cls = np.arange(8, dtype=np.int32)[None, :, None, None, None]
        oh = (lt == cls).astype(NPBF16)
        oh_prep = np.stack(
            [
                np.ascontiguousarray(oh[:, :4]).reshape(4, 128, 4096),
                np.ascontiguousarray(oh[:, 4:]).reshape(4, 128, 4096),
            ],
            axis=1,
        )

        l2 = lc.reshape(4, 2, 4, 32, 512)                # [sb, sh, sl, v1, v2]
        ll = np.ascontiguousarray(l2.transpose(2, 3, 0, 1, 4)).reshape(128, 4096)

        in_maps.append(
            {
                "x": x_prep,
                "onehot": oh_prep,
                "lablse": ll.astype(NPBF16),
                "g32": g32,
                "sel": sel,
                "ecol": ecol,
            }
        )
    return in_maps


def _combine(results):
    """Host gather: reduce per-core partials to the final scalar."""
    S_g = np.zeros(8, dtype=np.float64)
    S_lse = np.zeros(8, dtype=np.float64)
    cnt = np.zeros(8, dtype=np.float64)
    glse = 0.0
    n_total = 0
    for r in results:
        p = np.asarray(r["partials"], dtype=np.float64)
        glse += p[:, 0:8].sum()
        S_g += p[0:8, 8]
        cnt[:7] += p[0:7, 9]
        S_lse[:7] += p[0:7, 10]
        n_total += VOX_PER_CORE
    cnt[7] = n_total - cnt[:7].sum()
    S_lse[7] = glse - S_lse[:7].sum()
    sums = S_lse - S_g
    present = cnt > 0
    class_means = sums / np.maximum(cnt, 1.0)
    n_valid = present.sum()
    loss = np.where(present, class_means, 0.0).sum() / n_valid
    return np.float32(loss)


def run(inputs_maps=None, trace=False, **inputs):
    if "nc" not in _PROG_CACHE:
        _PROG_CACHE["nc"] = _build_program()
    nc = _PROG_CACHE["nc"]
    in_maps = inputs_maps if inputs_maps is not None else _host_prep(**inputs)
    res = bass_utils.run_bass_kernel_spmd(
        nc, in_maps, list(range(N_CORES)), trace=trace
    )
    return res


def kernel(output, labels):
    res = run(output=output, labels=labels)
    return _combine(res.results)


# revision 26
# speedup vs baseline: 1.1622x; 1.0270x over previous
"""Class-balanced cross-entropy loss kernel for Trainium2 (8 NeuronCores).

Problem: output [4,8,64,128,128] f32 logits, labels [4,1,64,128,128] int
(values 0..7).  loss = mean over present classes of (per-class mean CE).

Strategy (data-parallel over the flattened voxel axis, 524288 voxels/core):
  per-voxel CE loss  l_i = logsumexp_c(x_ic) - x_i[lab_i]
  per-class sums     sums[c]  = S_lse[c] - S_g[c]
     S_g[c]   = sum_{i: lab=c} x_i[c]
     S_lse[c] = sum_{i: lab=c} lse_i
     counts[c]
  final scalar combined on host from tiny per-core partials.

Inputs are pre-laid-out on host (sharding + one-hot label encoding only):
  x tiles   [4sb][2half][128, 4096] bf16, partition p = chat*32+v1,
            free f = shat*512+v2  (chat = class within half)
  onehot_x  same layout: 1.0 where lab == class(p) else 0
  lab_lse   [128, 4096] bf16 labels in the lse layout
Masked products run on the DVE at the 2x bf16 tensor_tensor rate; all
reductions run on the TensorEngine (PSUM-accumulating matmuls with
stationary selector matrices) or ride free activation accum_outs:
    s      = sum_c exp(x)      via G32 group-sum matmuls -> PSUM
    lse    = ln(s) on ACT (free accum_out -> global lse sum)
    S_g    = SEL^T @ (onehot_x * x)      accumulated in PSUM [8,512]
    S_lse  = E_c^T @ ((lab==c) * lse)    accumulated in PSUM [8,512]
    counts = E_c^T @ (lab==c)            accumulated in PSUM [8,512]
"""

import numpy as np
import ml_dtypes

import concourse.bass as bass
import concourse.bacc as bacc
import concourse.mybir as mybir
from concourse import bass_utils, tile

BF16 = mybir.dt.bfloat16
F32 = mybir.dt.float32
NPBF16 = ml_dtypes.bfloat16

N_CORES = 8
B, C, D, H, W = 4, 8, 64, 128, 128
N_SB = 4
SB_COLS = 4096
VOX_PER_CORE = 32 * H * W  # 524288

_PROG_CACHE = {}


def _build_program():
    nc = bacc.Bacc("TRN2", target_bir_lowering=False, debug=False)

    x_in = nc.dram_tensor("x", [N_SB, 2, 128, SB_COLS], BF16, kind="ExternalInput")
    oh_in = nc.dram_tensor("onehot", [N_SB, 2, 128, SB_COLS], BF16, kind="ExternalInput")
    ll_in = nc.dram_tensor("lablse", [128, SB_COLS], BF16, kind="ExternalInput")
    g32_in = nc.dram_tensor("g32", [128, 32], BF16, kind="ExternalInput")
    sel_in = nc.dram_tensor("sel", [128, 16], BF16, kind="ExternalInput")
    ecol_in = nc.dram_tensor("ecol", [128, 56], BF16, kind="ExternalInput")
    out_d = nc.dram_tensor("partials", [128, 11], F32, kind="ExternalOutput")

    eq = mybir.AluOpType.is_equal
    mul = mybir.AluOpType.mult

    with tile.TileContext(nc) as tc:
        with (
            tc.tile_pool(name="const", bufs=1) as cpool,
            tc.tile_pool(name="io", bufs=3) as iopool,
            tc.tile_pool(name="work", bufs=2) as wpool,
            tc.tile_pool(name="psum", bufs=4, space="PSUM") as ppool,
            tc.tile_pool(name="psacc", bufs=1, space="PSUM") as papool,
        ):
            # sb0's first half goes out before everything else
            xlo0 = iopool.tile([128, SB_COLS], BF16, tag="xlo")
            olo0 = iopool.tile([128, SB_COLS], BF16, tag="olo")
            xhi0 = iopool.tile([128, SB_COLS], BF16, tag="xhi")
            ohi0 = iopool.tile([128, SB_COLS], BF16, tag="ohi")
            h0 = slice(0, SB_COLS // 2)
            h1 = slice(SB_COLS // 2, SB_COLS)
            nc.sync.dma_start(xlo0[:, h0], x_in[0, 0][:, h0])
            nc.sync.dma_start(olo0[:, h0], oh_in[0, 0][:, h0])

            g32 = cpool.tile([128, 32], BF16)
            nc.sync.dma_start(g32[:], g32_in[:])
            sel = cpool.tile([128, 16], BF16)
            nc.sync.dma_start(sel[:], sel_in[:])
            ecol = cpool.tile([128, 56], BF16)
            nc.sync.dma_start(ecol[:], ecol_in[:])

            nc.sync.dma_start(xhi0[:, h0], x_in[0, 1][:, h0])
            nc.sync.dma_start(ohi0[:, h0], oh_in[0, 1][:, h0])
            nc.sync.dma_start(xlo0[:, h1], x_in[0, 0][:, h1])
            nc.sync.dma_start(olo0[:, h1], oh_in[0, 0][:, h1])
            nc.sync.dma_start(xhi0[:, h1], x_in[0, 1][:, h1])
            nc.sync.dma_start(ohi0[:, h1], oh_in[0, 1][:, h1])

            lab_lse = cpool.tile([128, SB_COLS], BF16)
            nc.sync.dma_start(lab_lse[:], ll_in[:])
            lse = cpool.tile([128, SB_COLS], BF16)
            glse_acc = cpool.tile([128, 8], F32)
            final = cpool.tile([8, 3], F32)

            # psum accumulators alive for the whole kernel
            ps_sg = papool.tile([8, 512], F32)
            ps_cnt = papool.tile([8, 512], F32)
            ps_slse = papool.tile([8, 512], F32)

            # tiny reads that absorb DMA semaphore waits
            dummy = cpool.tile([128, 4], F32)
            nc.vector.tensor_copy(dummy[:, 0:1], g32[:, 0:1])
            nc.vector.tensor_copy(dummy[:, 2:3], sel[:, 0:1])

            first_sg = [True]
            first_cls = [True]

            def sg_matmuls(m, h, cols):
                lhs = sel[:, 8 * h : 8 * h + 8]
                for ci in range(cols.start // 512, cols.stop // 512):
                    nc.tensor.matmul(
                        ps_sg[:, :],
                        lhs,
                        m[:, 512 * ci : 512 * (ci + 1)],
                        start=first_sg[0],
                        stop=False,
                        skip_group_check=True,
                    )
                    first_sg[0] = False

            pstiles = []
            for sb in range(N_SB):
                if sb == 0:
                    xlo, xhi, olo, ohi = xlo0, xhi0, olo0, ohi0
                    chunks = (h0, h1)
                else:
                    xlo = iopool.tile([128, SB_COLS], BF16, tag="xlo")
                    olo = iopool.tile([128, SB_COLS], BF16, tag="olo")
                    xhi = iopool.tile([128, SB_COLS], BF16, tag="xhi")
                    ohi = iopool.tile([128, SB_COLS], BF16, tag="ohi")
                    nc.sync.dma_start(xlo[:], x_in[sb, 0])
                    nc.sync.dma_start(olo[:], oh_in[sb, 0])
                    nc.sync.dma_start(xhi[:], x_in[sb, 1])
                    nc.sync.dma_start(ohi[:], oh_in[sb, 1])
                    chunks = (slice(0, SB_COLS),)

                nc.vector.tensor_copy(dummy[:, 0:1], olo[:, 0:1])
                nc.vector.tensor_copy(dummy[:, 1:2], ohi[:, 0:1])

                elo = wpool.tile([128, SB_COLS], BF16, tag="elo")
                ehi = wpool.tile([128, SB_COLS], BF16, tag="ehi")
                mlo = wpool.tile([128, SB_COLS], BF16, tag="mlo")
                mhi = wpool.tile([128, SB_COLS], BF16, tag="mhi")
                for cs in chunks:
                    nc.vector.tensor_tensor(mlo[:, cs], olo[:, cs], xlo[:, cs], mul)
                    sg_matmuls(mlo, 0, cs)
                    nc.scalar.activation(
                        elo[:, cs], xlo[:, cs], mybir.ActivationFunctionType.Exp
                    )
                    nc.vector.tensor_tensor(mhi[:, cs], ohi[:, cs], xhi[:, cs], mul)
                    sg_matmuls(mhi, 1, cs)
                    nc.scalar.activation(
                        ehi[:, cs], xhi[:, cs], mybir.ActivationFunctionType.Exp
                    )

                # softmax denominator: class-group sums on PE
                for g in range(2):
                    ps = ppool.tile([128, 512], F32, tag="ps")
                    for q in range(4):
                        sl = 512 * (4 * g + q)
                        nc.tensor.matmul(
                            ps[32 * q : 32 * (q + 1), :],
                            g32[:],
                            elo[:, sl : sl + 512],
                            start=True,
                            stop=False,
                            tile_position=(0, 32 * q),
                        )
                        nc.tensor.matmul(
                            ps[32 * q : 32 * (q + 1), :],
                            g32[:],
                            ehi[:, sl : sl + 512],
                            start=False,
                            stop=True,
                            tile_position=(0, 32 * q),
                        )
                    pstiles.append((2 * sb + g, ps))

                # lns (fused global-lse accum), then per-class masked lse
                # products + counts on this sb's [128, 1024] slice, PE-reduced
                if sb == 0:
                    # absorb waits for the class-pass constants off the
                    # early critical path
                    nc.vector.tensor_copy(dummy[:, 1:2], lab_lse[:, 0:1])
                    nc.vector.tensor_copy(dummy[:, 3:4], ecol[:, 0:1])
                for u, ps in pstiles:
                    nc.scalar.activation(
                        lse[:, 512 * u : 512 * (u + 1)],
                        ps[:],
                        mybir.ActivationFunctionType.Ln,
                        accum_out=glse_acc[:, u : u + 1],
                    )
                pstiles = []
                pcs = slice(1024 * sb, 1024 * (sb + 1))
                for c in range(7):
                    ohc = wpool.tile([128, 1024], BF16, tag="ohc")
                    nc.vector.tensor_scalar(
                        ohc[:], lab_lse[:, pcs], float(c), None, eq
                    )
                    mls = wpool.tile([128, 1024], BF16, tag="mls")
                    nc.vector.tensor_tensor(mls[:], ohc[:], lse[:, pcs], mul)
                    lhs = ecol[:, 8 * c : 8 * c + 8]
                    for ci in range(2):
                        nc.tensor.matmul(
                            ps_cnt[:, :],
                            lhs,
                            ohc[:, 512 * ci : 512 * (ci + 1)],
                            start=first_cls[0],
                            stop=False,
                            skip_group_check=True,
                        )
                        nc.tensor.matmul(
                            ps_slse[:, :],
                            lhs,
                            mls[:, 512 * ci : 512 * (ci + 1)],
                            start=first_cls[0],
                            stop=False,
                            skip_group_check=True,
                        )
                        first_cls[0] = False

            # fold the [8, 512] psum accumulators to [8, 1]
            nc.vector.tensor_reduce(
                final[0:8, 0:1], ps_sg[:], mybir.AxisListType.X, mybir.AluOpType.add
            )
            nc.vector.tensor_reduce(
                final[0:8, 1:2], ps_cnt[:], mybir.AxisListType.X, mybir.AluOpType.add
            )
            nc.vector.tensor_reduce(
                final[0:8, 2:3], ps_slse[:], mybir.AxisListType.X, mybir.AluOpType.add
            )

            nc.sync.dma_start(out_d[:, 0:8], glse_acc[:])
            nc.sync.dma_start(out_d[0:8, 8:11], final[0:8, 0:3])

    nc.compile()
    return nc


def _host_prep(output, labels):
    """Build per-core input maps (sharding + layout/encoding prep)."""
    x = np.asarray(output)
    lab = np.asarray(labels).astype(np.int32)

    g32 = np.zeros((128, 32), dtype=NPBF16)
    for ch in range(4):
        for v1 in range(32):
            g32[ch * 32 + v1, v1] = 1.0
    sel = np.zeros((128, 16), dtype=NPBF16)
    for p in range(128):
        sel[p, p // 32] = 1.0            # lo half -> classes 0..3
        sel[p, 8 + 4 + p // 32] = 1.0    # hi half -> classes 4..7
    ecol = np.zeros((128, 56), dtype=NPBF16)
    for c in range(7):
        ecol[:, 8 * c + c] = 1.0

    in_maps = []
    for k in range(N_CORES):
        b, d0 = k // 2, 32 * (k % 2)
        xc = x[b, :, d0 : d0 + 32].reshape(8, 4, 8, 32, 512)
        xt = xc.transpose(1, 0, 3, 2, 4).astype(NPBF16)  # [sb, c, v1, shat, v2]
        x_prep = np.stack(
            [
                np.ascontiguousarray(xt[:, :4]).reshape(4, 128, 4096),
                np.ascontiguousarray(xt[:, 4:]).reshape(4, 128, 4096),
            ],
            axis=1,
        )

        lc = lab[b, 0, d0 : d0 + 32].reshape(4, 8, 32, 512)
        # one-hot label encoding in the x layout: [sb, cls, v1, shat, v2]
        lt = lc.transpose(0, 2, 1, 3)[:, None]           # [sb, 1, v1, shat, v2]
        cls = np.arange(8, dtype=np.int32)[None, :, None, None, None]
        oh = (lt == cls).astype(NPBF16)
        oh_prep = np.stack(
            [
                np.ascontiguousarray(oh[:, :4]).reshape(4, 128, 4096),
                np.ascontiguousarray(oh[:, 4:]).reshape(4, 128, 4096),
            ],
            axis=1,
        )

        l2 = lc.reshape(4, 2, 4, 32, 512)                # [sb, sh, sl, v1, v2]
        ll = np.ascontiguousarray(l2.transpose(2, 3, 0, 1, 4)).reshape(128, 4096)

        in_maps.append(
            {
                "x": x_prep,
                "onehot": oh_prep,
                "lablse": ll.astype(NPBF16),
                "g32": g32,
                "sel": sel,
                "ecol": ecol,
            }
        )
    return in_maps


def _combine(results):
    """Host gather: reduce per-core partials to the final scalar."""
    S_g = np.zeros(8, dtype=np.float64)
    S_lse = np.zeros(8, dtype=np.float64)
    cnt = np.zeros(8, dtype=np.float64)
    glse = 0.0
    n_total = 0
    for r in results:
        p = np.asarray(r["partials"], dtype=np.float64)
        glse += p[:, 0:8].sum()
        S_g += p[0:8, 8]
        cnt[:7] += p[0:7, 9]
        S_lse[:7] += p[0:7, 10]
        n_total += VOX_PER_CORE
    cnt[7] = n_total - cnt[:7].sum()
    S_lse[7] = glse - S_lse[:7].sum()
    sums = S_lse - S_g
    present = cnt > 0
    class_means = sums / np.maximum(cnt, 1.0)
    n_valid = present.sum()
    loss = np.where(present, class_means, 0.0).sum() / n_valid
    return np.float32(loss)


def run(inputs_maps=None, trace=False, **inputs):
    if "nc" not in _PROG_CACHE:
        _PROG_CACHE["nc"] = _build_program()
    nc = _PROG_CACHE["nc"]
    in_maps = inputs_maps if inputs_maps is not None else _host_prep(**inputs)
    res = bass_utils.run_bass_kernel_spmd(
        nc, in_maps, list(range(N_CORES)), trace=trace
    )
    return res


def kernel(output, labels):
    res = run(output=output, labels=labels)
    return _combine(res.results)


# revision 27
# speedup vs baseline: 1.1761x; 1.0119x over previous
"""Class-balanced cross-entropy loss kernel for Trainium2 (8 NeuronCores).

Problem: output [4,8,64,128,128] f32 logits, labels [4,1,64,128,128] int
(values 0..7).  loss = mean over present classes of (per-class mean CE).

Strategy (data-parallel over the flattened voxel axis, 524288 voxels/core):
  per-voxel CE loss  l_i = logsumexp_c(x_ic) - x_i[lab_i]
  per-class sums     sums[c]  = S_lse[c] - S_g[c]
     S_g[c]   = sum_{i: lab=c} x_i[c]
     S_lse[c] = sum_{i: lab=c} lse_i
     counts[c]
  final scalar combined on host from tiny per-core partials.

Inputs are pre-laid-out on host (sharding + one-hot label encoding only):
  x tiles   [4sb][2half][128, 4096] bf16, partition p = chat*32+v1,
            free f = shat*512+v2  (chat = class within half)
  onehot_x  same layout: 1.0 where lab == class(p) else 0
  lab_lse   [128, 4096] bf16 labels in the lse layout
Masked products run on the DVE at the 2x bf16 tensor_tensor rate; all
reductions run on the TensorEngine (PSUM-accumulating matmuls with
stationary selector matrices) or ride free activation accum_outs:
    s      = sum_c exp(x)      via G32 group-sum matmuls -> PSUM
    lse    = ln(s) on ACT (free accum_out -> global lse sum)
    S_g    = SEL^T @ (onehot_x * x)      accumulated in PSUM [8,512]
    S_lse  = E_c^T @ ((lab==c) * lse)    accumulated in PSUM [8,512]
    counts = E_c^T @ (lab==c)            accumulated in PSUM [8,512]
"""

import numpy as np
import ml_dtypes

import concourse.bass as bass
import concourse.bacc as bacc
import concourse.mybir as mybir
from concourse import bass_utils, tile

BF16 = mybir.dt.bfloat16
F32 = mybir.dt.float32
NPBF16 = ml_dtypes.bfloat16

N_CORES = 8
B, C, D, H, W = 4, 8, 64, 128, 128
N_SB = 4
SB_COLS = 4096
VOX_PER_CORE = 32 * H * W  # 524288

_PROG_CACHE = {}


def _build_program():
    nc = bacc.Bacc("TRN2", target_bir_lowering=False, debug=False)

    x_in = nc.dram_tensor("x", [N_SB, 2, 128, SB_COLS], BF16, kind="ExternalInput")
    oh_in = nc.dram_tensor("onehot", [N_SB, 2, 128, SB_COLS], BF16, kind="ExternalInput")
    ll_in = nc.dram_tensor("lablse", [128, SB_COLS], BF16, kind="ExternalInput")
    g32_in = nc.dram_tensor("g32", [128, 32], BF16, kind="ExternalInput")
    sel_in = nc.dram_tensor("sel", [128, 16], BF16, kind="ExternalInput")
    ecol_in = nc.dram_tensor("ecol", [128, 56], BF16, kind="ExternalInput")
    out_d = nc.dram_tensor("partials", [128, 11], F32, kind="ExternalOutput")

    eq = mybir.AluOpType.is_equal
    mul = mybir.AluOpType.mult

    with tile.TileContext(nc) as tc:
        with (
            tc.tile_pool(name="const", bufs=1) as cpool,
            tc.tile_pool(name="io", bufs=3) as iopool,
            tc.tile_pool(name="work", bufs=2) as wpool,
            tc.tile_pool(name="psum", bufs=4, space="PSUM") as ppool,
            tc.tile_pool(name="psacc", bufs=1, space="PSUM") as papool,
        ):
            # sb0's first half goes out before everything else
            xlo0 = iopool.tile([128, SB_COLS], BF16, tag="xlo")
            olo0 = iopool.tile([128, SB_COLS], BF16, tag="olo")
            xhi0 = iopool.tile([128, SB_COLS], BF16, tag="xhi")
            ohi0 = iopool.tile([128, SB_COLS], BF16, tag="ohi")
            h0 = slice(0, SB_COLS // 2)
            h1 = slice(SB_COLS // 2, SB_COLS)
            nc.sync.dma_start(xlo0[:, h0], x_in[0, 0][:, h0])
            nc.sync.dma_start(olo0[:, h0], oh_in[0, 0][:, h0])

            g32 = cpool.tile([128, 32], BF16)
            nc.sync.dma_start(g32[:], g32_in[:])
            sel = cpool.tile([128, 16], BF16)
            nc.sync.dma_start(sel[:], sel_in[:])
            ecol = cpool.tile([128, 56], BF16)
            nc.sync.dma_start(ecol[:], ecol_in[:])

            nc.sync.dma_start(xhi0[:, h0], x_in[0, 1][:, h0])
            nc.sync.dma_start(ohi0[:, h0], oh_in[0, 1][:, h0])
            nc.sync.dma_start(xlo0[:, h1], x_in[0, 0][:, h1])
            nc.sync.dma_start(olo0[:, h1], oh_in[0, 0][:, h1])
            nc.sync.dma_start(xhi0[:, h1], x_in[0, 1][:, h1])
            nc.sync.dma_start(ohi0[:, h1], oh_in[0, 1][:, h1])

            lab_lse = cpool.tile([128, SB_COLS], BF16)
            nc.sync.dma_start(lab_lse[:], ll_in[:])
            lse = cpool.tile([128, SB_COLS], BF16)
            glse_acc = cpool.tile([128, 8], F32)
            final = cpool.tile([8, 3], F32)

            # psum accumulators alive for the whole kernel
            ps_sg = papool.tile([8, 512], F32)
            ps_cnt = papool.tile([8, 512], F32)
            ps_slse = papool.tile([8, 512], F32)

            # tiny reads that absorb DMA semaphore waits
            dummy = cpool.tile([128, 4], F32)
            nc.vector.tensor_copy(dummy[:, 0:1], g32[:, 0:1])
            nc.vector.tensor_copy(dummy[:, 2:3], sel[:, 0:1])

            first_sg = [True]
            first_cls = [True]

            def sg_matmuls(m, h, cols):
                lhs = sel[:, 8 * h : 8 * h + 8]
                for ci in range(cols.start // 512, cols.stop // 512):
                    nc.tensor.matmul(
                        ps_sg[:, :],
                        lhs,
                        m[:, 512 * ci : 512 * (ci + 1)],
                        start=first_sg[0],
                        stop=False,
                        skip_group_check=True,
                    )
                    first_sg[0] = False

            pstiles = []
            for sb in range(N_SB):
                if sb == 0:
                    xlo, xhi, olo, ohi = xlo0, xhi0, olo0, ohi0
                    chunks = (h0, h1)
                else:
                    xlo = iopool.tile([128, SB_COLS], BF16, tag="xlo")
                    olo = iopool.tile([128, SB_COLS], BF16, tag="olo")
                    xhi = iopool.tile([128, SB_COLS], BF16, tag="xhi")
                    ohi = iopool.tile([128, SB_COLS], BF16, tag="ohi")
                    if sb == N_SB - 1:
                        for cc in (h0, h1):
                            nc.sync.dma_start(xlo[:, cc], x_in[sb, 0][:, cc])
                            nc.sync.dma_start(olo[:, cc], oh_in[sb, 0][:, cc])
                            nc.sync.dma_start(xhi[:, cc], x_in[sb, 1][:, cc])
                            nc.sync.dma_start(ohi[:, cc], oh_in[sb, 1][:, cc])
                        chunks = (h0, h1)
                    else:
                        nc.sync.dma_start(xlo[:], x_in[sb, 0])
                        nc.sync.dma_start(olo[:], oh_in[sb, 0])
                        nc.sync.dma_start(xhi[:], x_in[sb, 1])
                        nc.sync.dma_start(ohi[:], oh_in[sb, 1])
                        chunks = (slice(0, SB_COLS),)

                nc.vector.tensor_copy(dummy[:, 0:1], olo[:, 0:1])
                nc.vector.tensor_copy(dummy[:, 1:2], ohi[:, 0:1])

                elo = wpool.tile([128, SB_COLS], BF16, tag="elo")
                ehi = wpool.tile([128, SB_COLS], BF16, tag="ehi")
                mlo = wpool.tile([128, SB_COLS], BF16, tag="mlo")
                mhi = wpool.tile([128, SB_COLS], BF16, tag="mhi")
                for cs in chunks:
                    nc.vector.tensor_tensor(mlo[:, cs], olo[:, cs], xlo[:, cs], mul)
                    sg_matmuls(mlo, 0, cs)
                    nc.scalar.activation(
                        elo[:, cs], xlo[:, cs], mybir.ActivationFunctionType.Exp
                    )
                    nc.vector.tensor_tensor(mhi[:, cs], ohi[:, cs], xhi[:, cs], mul)
                    sg_matmuls(mhi, 1, cs)
                    nc.scalar.activation(
                        ehi[:, cs], xhi[:, cs], mybir.ActivationFunctionType.Exp
                    )

                # softmax denominator: class-group sums on PE
                for g in range(2):
                    ps = ppool.tile([128, 512], F32, tag="ps")
                    for q in range(4):
                        sl = 512 * (4 * g + q)
                        nc.tensor.matmul(
                            ps[32 * q : 32 * (q + 1), :],
                            g32[:],
                            elo[:, sl : sl + 512],
                            start=True,
                            stop=False,
                            tile_position=(0, 32 * q),
                        )
                        nc.tensor.matmul(
                            ps[32 * q : 32 * (q + 1), :],
                            g32[:],
                            ehi[:, sl : sl + 512],
                            start=False,
                            stop=True,
                            tile_position=(0, 32 * q),
                        )
                    pstiles.append((2 * sb + g, ps))

                # lns (fused global-lse accum), then per-class masked lse
                # products + counts on this sb's [128, 1024] slice, PE-reduced
                if sb == 0:
                    # absorb waits for the class-pass constants off the
                    # early critical path
                    nc.vector.tensor_copy(dummy[:, 1:2], lab_lse[:, 0:1])
                    nc.vector.tensor_copy(dummy[:, 3:4], ecol[:, 0:1])
                for u, ps in pstiles:
                    nc.scalar.activation(
                        lse[:, 512 * u : 512 * (u + 1)],
                        ps[:],
                        mybir.ActivationFunctionType.Ln,
                        accum_out=glse_acc[:, u : u + 1],
                    )
                pstiles = []
                if sb == N_SB - 1:
                    spans = [
                        slice(1024 * sb, 1024 * sb + 512),
                        slice(1024 * sb + 512, 1024 * (sb + 1)),
                    ]
                else:
                    spans = [slice(1024 * sb, 1024 * (sb + 1))]
                for pcs in spans:
                    w = pcs.stop - pcs.start
                    for c in range(7):
                        ohc = wpool.tile([128, 1024], BF16, tag="ohc")
                        nc.vector.tensor_scalar(
                            ohc[:, 0:w], lab_lse[:, pcs], float(c), None, eq
                        )
                        mls = wpool.tile([128, 1024], BF16, tag="mls")
                        nc.vector.tensor_tensor(
                            mls[:, 0:w], ohc[:, 0:w], lse[:, pcs], mul
                        )
                        lhs = ecol[:, 8 * c : 8 * c + 8]
                        for ci in range(w // 512):
                            nc.tensor.matmul(
                                ps_cnt[:, :],
                                lhs,
                                ohc[:, 512 * ci : 512 * (ci + 1)],
                                start=first_cls[0],
                                stop=False,
                                skip_group_check=True,
                            )
                            nc.tensor.matmul(
                                ps_slse[:, :],
                                lhs,
                                mls[:, 512 * ci : 512 * (ci + 1)],
                                start=first_cls[0],
                                stop=False,
                                skip_group_check=True,
                            )
                            first_cls[0] = False

            # fold the [8, 512] psum accumulators to [8, 1]
            nc.vector.tensor_reduce(
                final[0:8, 0:1], ps_sg[:], mybir.AxisListType.X, mybir.AluOpType.add
            )
            nc.vector.tensor_reduce(
                final[0:8, 1:2], ps_cnt[:], mybir.AxisListType.X, mybir.AluOpType.add
            )
            nc.vector.tensor_reduce(
                final[0:8, 2:3], ps_slse[:], mybir.AxisListType.X, mybir.AluOpType.add
            )

            nc.sync.dma_start(out_d[:, 0:8], glse_acc[:])
            nc.sync.dma_start(out_d[0:8, 8:11], final[0:8, 0:3])

    nc.compile()
    return nc


def _host_prep(output, labels):
    """Build per-core input maps (sharding + layout/encoding prep)."""
    x = np.asarray(output)
    lab = np.asarray(labels).astype(np.int32)

    g32 = np.zeros((128, 32), dtype=NPBF16)
    for ch in range(4):
        for v1 in range(32):
            g32[ch * 32 + v1, v1] = 1.0
    sel = np.zeros((128, 16), dtype=NPBF16)
    for p in range(128):
        sel[p, p // 32] = 1.0            # lo half -> classes 0..3
        sel[p, 8 + 4 + p // 32] = 1.0    # hi half -> classes 4..7
    ecol = np.zeros((128, 56), dtype=NPBF16)
    for c in range(7):
        ecol[:, 8 * c + c] = 1.0

    in_maps = []
    for k in range(N_CORES):
        b, d0 = k // 2, 32 * (k % 2)
        xc = x[b, :, d0 : d0 + 32].reshape(8, 4, 8, 32, 512)
        xt = xc.transpose(1, 0, 3, 2, 4).astype(NPBF16)  # [sb, c, v1, shat, v2]
        x_prep = np.stack(
            [
                np.ascontiguousarray(xt[:, :4]).reshape(4, 128, 4096),
                np.ascontiguousarray(xt[:, 4:]).reshape(4, 128, 4096),
            ],
            axis=1,
        )

        lc = lab[b, 0, d0 : d0 + 32].reshape(4, 8, 32, 512)
        # one-hot label encoding in the x layout: [sb, cls, v1, shat, v2]
        lt = lc.transpose(0, 2, 1, 3)[:, None]           # [sb, 1, v1, shat, v2]
        cls = np.arange(8, dtype=np.int32)[None, :, None, None, None]
        oh = (lt == cls).astype(NPBF16)
        oh_prep = np.stack(
            [
                np.ascontiguousarray(oh[:, :4]).reshape(4, 128, 4096),
                np.ascontiguousarray(oh[:, 4:]).reshape(4, 128, 4096),
            ],
            axis=1,
        )

        l2 = lc.reshape(4, 2, 4, 32, 512)                # [sb, sh, sl, v1, v2]
        ll = np.ascontiguousarray(l2.transpose(2, 3, 0, 1, 4)).reshape(128, 4096)

        in_maps.append(
            {
                "x": x_prep,
                "onehot": oh_prep,
                "lablse": ll.astype(NPBF16),
                "g32": g32,
                "sel": sel,
                "ecol": ecol,
            }
        )
    return in_maps


def _combine(results):
    """Host gather: reduce per-core partials to the final scalar."""
    S_g = np.zeros(8, dtype=np.float64)
    S_lse = np.zeros(8, dtype=np.float64)
    cnt = np.zeros(8, dtype=np.float64)
    glse = 0.0
    n_total = 0
    for r in results:
        p = np.asarray(r["partials"], dtype=np.float64)
        glse += p[:, 0:8].sum()
        S_g += p[0:8, 8]
        cnt[:7] += p[0:7, 9]
        S_lse[:7] += p[0:7, 10]
        n_total += VOX_PER_CORE
    cnt[7] = n_total - cnt[:7].sum()
    S_lse[7] = glse - S_lse[:7].sum()
    sums = S_lse - S_g
    present = cnt > 0
    class_means = sums / np.maximum(cnt, 1.0)
    n_valid = present.sum()
    loss = np.where(present, class_means, 0.0).sum() / n_valid
    return np.float32(loss)


def run(inputs_maps=None, trace=False, **inputs):
    if "nc" not in _PROG_CACHE:
        _PROG_CACHE["nc"] = _build_program()
    nc = _PROG_CACHE["nc"]
    in_maps = inputs_maps if inputs_maps is not None else _host_prep(**inputs)
    res = bass_utils.run_bass_kernel_spmd(
        nc, in_maps, list(range(N_CORES)), trace=trace
    )
    return res


def kernel(output, labels):
    res = run(output=output, labels=labels)
    return _combine(res.results)
